# Initial kernel scaffold
#
"""Chamfer loss Trainium2 kernel.

Problem: x, y of shape (N=8, V=4096, C=3) fp32. Loss =
mean_n [ mean_w min_v ||x_nv - y_nw|| + mean_v min_w ||x_nv - y_nw|| ].

Sharding: one batch per NeuronCore (data parallel over N=8), 8 cores.

Per-core algorithm:
  - Host packs augmented matrices AX, AY [13, 4096] bf16 implementing an
    error-compensated (hi/lo split) gram expansion so that
    sq[v, w] = sum_k AX[k, v] * AY[k, w] = ||x_v||^2 + ||y_w||^2 - 2 x_v.y_w
    to ~fp32 accuracy while running the PE at bf16 speed (1 col/cycle).
  - PE computes sq in 32x8 tiles of [128, 512] fp32 in PSUM.
  - DVE consumes each [128, 2048] PSUM group twice:
      * row-direction: running elementwise min + fused final reduce
        (tensor_tensor_reduce) -> min_w sq[v, :] for each v.
      * col-direction: running elementwise min into cacc[g] [128, 2048].
  - Epilogue: PE-transpose cacc blocks, reduce -> min_v sq[:, w] per w;
    relu, sqrt (ACT), sum (DVE), partition-sum via PE ones-matvec; scale.
  - Output: per-core scalar partial loss; host averages the 8 partials.
"""

import sys

sys.path.insert(0, "/opt/trn_rl_repo")

from contextlib import ExitStack

import ml_dtypes
import numpy as np

import concourse.bass as bass
import concourse.tile as tile
from concourse import mybir
from concourse.bass_utils import run_bass_kernel_spmd

BF16 = ml_dtypes.bfloat16

P = 128  # partitions / m-block size
V = 4096  # points per batch
KA = 13  # augmented contraction dim
NMM = 512  # matmul free dim (one PSUM bank fp32)
GRP = 2048  # PSUM group free dim (4 banks)
NG = V // GRP  # 2 groups per m-block
MB = V // P  # 32 m-blocks
BIG = 3.0e38

_cache = {}


def _build_nc():
    F32 = mybir.dt.float32
    B16 = mybir.dt.bfloat16
    mn = mybir.AluOpType.min

    nc = bass.Bass()
    ax_d = nc.declare_dram_parameter("ax", [KA, V], B16, isOutput=False)
    ay_d = nc.declare_dram_parameter("ay", [KA, V], B16, isOutput=False)
    id_d = nc.declare_dram_parameter("ident", [P, P], F32, isOutput=False)
    ones_d = nc.declare_dram_parameter("ones", [P, 1], F32, isOutput=False)
    loss_d = nc.declare_dram_parameter("loss", [1, 1], F32, isOutput=True)

    with tile.TileContext(nc) as tc, ExitStack() as ctx:
        const = ctx.enter_context(tc.tile_pool(name="const", bufs=1))
        accs = ctx.enter_context(tc.tile_pool(name="accs", bufs=1))

        ax_sb = const.tile([KA, V], B16)
        ay_sb = const.tile([KA, V], B16)
        id_sb = const.tile([P, P], F32)
        ones_sb = const.tile([P, 1], F32)
        nc.sync.dma_start(ax_sb[:], ax_d[:])
        nc.sync.dma_start(ay_sb[:], ay_d[:])
        nc.sync.dma_start(id_sb[:], id_d[:])
        nc.sync.dma_start(ones_sb[:], ones_d[:])

        cacc = [accs.tile([P, GRP], F32, tag=f"cacc{g}") for g in range(NG)]
        racc = accs.tile([P, GRP], F32)
        rowmin = accs.tile([P, MB], F32)
        colmin = accs.tile([P, MB], F32)

        with tc.tile_pool(name="psum", bufs=2, space="PSUM") as psum:
            for m in range(MB):
                lhsT = ax_sb[:, m * P : (m + 1) * P]
                for g in range(NG):
                    ps = psum.tile([P, GRP], F32)
                    for j in range(GRP // NMM):
                        c0 = g * GRP + j * NMM
                        nc.tensor.matmul(
                            ps[:, j * NMM : (j + 1) * NMM],
                            lhsT,
                            ay_sb[:, c0 : c0 + NMM],
                            start=True,
                            stop=True,
                        )
                    # col-direction (min over v): running min across m-blocks
                    if m == 0:
                        nc.vector.tensor_copy(cacc[g][:], ps[:])
                    else:
                        nc.vector.tensor_tensor(cacc[g][:], ps[:], cacc[g][:], mn)
                    # row-direction (min over w): chain across groups; the
                    # last group fuses the full [P, GRP] reduce.
                    if g == 0:
                        nc.vector.tensor_copy(racc[:], ps[:])
                    else:
                        nc.vector.tensor_tensor_reduce(
                            out=racc[:],
                            in0=ps[:],
                            in1=racc[:],
                            scale=1.0,
                            scalar=BIG,
                            op0=mn,
                            op1=mn,
                            accum_out=rowmin[:, m : m + 1],
                        )

        # Epilogue: transpose cacc -> per-w mins.
        with tc.tile_pool(name="psum_ep", bufs=4, space="PSUM") as psum_ep:
            for g in range(NG):
                for b in range(GRP // P):
                    tp = psum_ep.tile([P, P], F32)
                    nc.tensor.transpose(
                        tp[:], cacc[g][:, b * P : (b + 1) * P], id_sb[:]
                    )
                    idx = g * (GRP // P) + b
                    nc.vector.tensor_reduce(
                        colmin[:, idx : idx + 1], tp[:], axis=mybir.AxisListType.X, op=mn
                    )

            # relu (negative sq from roundoff) then sqrt -> distances.
            nc.vector.tensor_scalar_max(rowmin[:], rowmin[:], 0.0)
            nc.vector.tensor_scalar_max(colmin[:], colmin[:], 0.0)
            nc.scalar.sqrt(rowmin[:], rowmin[:])
            nc.scalar.sqrt(colmin[:], colmin[:])

            sa = accs.tile([P, 1], F32, tag="sa")
            sb_ = accs.tile([P, 1], F32, tag="sb")
            stot = accs.tile([P, 1], F32, tag="stot")
            nc.vector.tensor_reduce(
                sa[:], rowmin[:], axis=mybir.AxisListType.X, op=mybir.AluOpType.add
            )
            nc.vector.tensor_reduce(
                sb_[:], colmin[:], axis=mybir.AxisListType.X, op=mybir.AluOpType.add
            )
            nc.vector.tensor_add(stot[:], sa[:], sb_[:])

            fin = psum_ep.tile([1, 1], F32, tag="fin")
            nc.tensor.matmul(fin[:], stot[:], ones_sb[:], start=True, stop=True)
            res = accs.tile([1, 1], F32, tag="res")
            nc.scalar.mul(res[:], fin[:], 1.0 / V)
            nc.sync.dma_start(loss_d[:], res[:])

    return nc


def _augment(x, y):
    """x, y: (V, 3) fp32 -> AX, AY [13, V] bf16 hi/lo-split gram operands."""
    f32 = np.float32
    yy = (-2.0 * y).astype(f32)
    xh = x.astype(BF16)
    xl = (x - xh.astype(f32)).astype(BF16)
    yh = yy.astype(BF16)
    yl = (yy - yh.astype(f32)).astype(BF16)
    x2 = np.einsum("vc,vc->v", x.astype(np.float64), x.astype(np.float64)).astype(f32)
    y2 = np.einsum("vc,vc->v", y.astype(np.float64), y.astype(np.float64)).astype(f32)
    x2h = x2.astype(BF16)
    x2l = (x2 - x2h.astype(f32)).astype(BF16)
    y2h = y2.astype(BF16)
    y2l = (y2 - y2h.astype(f32)).astype(BF16)
    one = np.ones(V, dtype=BF16)

    ax = np.stack(
        [xh[:, 0], xh[:, 1], xh[:, 2],
         xh[:, 0], xh[:, 1], xh[:, 2],
         xl[:, 0], xl[:, 1], xl[:, 2],
         x2h, x2l, one, one]
    )
    ay = np.stack(
        [yh[:, 0], yh[:, 1], yh[:, 2],
         yl[:, 0], yl[:, 1], yl[:, 2],
         yh[:, 0], yh[:, 1], yh[:, 2],
         one, one, y2h, y2l]
    )
    return ax, ay


def kernel(x, y):
    x = np.asarray(x, dtype=np.float32)
    y = np.asarray(y, dtype=np.float32)
    n = x.shape[0]
    assert x.shape == (n, V, 3) and y.shape == (n, V, 3) and n == 8

    if "nc" not in _cache:
        _cache["nc"] = _build_nc()
    nc = _cache["nc"]

    ident = np.eye(P, dtype=np.float32)
    ones = np.ones((P, 1), dtype=np.float32)
    in_maps = []
    for i in range(n):
        ax, ay = _augment(x[i], y[i])
        in_maps.append({"ax": ax, "ay": ay, "ident": ident, "ones": ones})

    res = run_bass_kernel_spmd(nc, in_maps, list(range(n)))
    vals = [np.asarray(res.results[i]["loss"], dtype=np.float32).reshape(()) for i in range(n)]
    return np.float32(np.mean(vals))


# revision 5
# speedup vs baseline: 1.0360x; 1.0360x over previous
"""Chamfer loss Trainium2 kernel (data-parallel over batch, 8 NeuronCores).

Problem: x, y (8, 4096, 3) fp32; loss = mean_n [ mean_w min_v ||x_nv - y_nw||
+ mean_v min_w ||x_nv - y_nw|| ] (scalar fp32).

Per core (one batch):
  - Host packs augmented operands AX, AY [13, 4096] bf16 via an
    error-compensated hi/lo split so the PE gram matmul produces
    sq[v,w] = ||x_v||^2 + ||y_w||^2 - 2 x_v.y_w at ~fp32 accuracy while
    streaming at bf16 rate (1 col/cycle).
  - PE: 32 m-blocks x 8 matmuls of [13,128]^T @ [13,512] -> PSUM
    [128, 2048] groups (4 banks, double buffered).
  - ACT (ScalarE): evacuates each PSUM group to SBUF fp16 (the only other
    engine that can read PSUM; runs in parallel with DVE).
  - DVE: row-direction min (min over w per v): fp16 2x-mode fold tree
    per m-block + one batched strided reduce per 4 m-blocks;
    col-direction min (min over v per w): fp16 running min chains into
    two [128, 2048] accumulators.
  - Epilogue: PE-transposes of the col accumulators + strided reduces
    -> per-w mins; relu, ACT sqrt with fused free-dim sum accumulation,
    partition sum via PE ones-matvec, scale by 1/V; DMA scalar out.
  - Host averages the 8 per-core partial losses.
"""

import sys

sys.path.insert(0, "/opt/trn_rl_repo")

from contextlib import ExitStack

import ml_dtypes
import numpy as np

import concourse.bacc as bacc
import concourse.tile as tile
from concourse import mybir
from concourse.bass_utils import run_bass_kernel_spmd

BF16 = ml_dtypes.bfloat16

P = 128
V = 4096
KA = 13  # augmented contraction dim
NMM = 512  # matmul moving free dim (one fp32 PSUM bank)
GRP = 2048  # PSUM group (4 banks)
NG = V // GRP  # 2 groups per m-block
MB = V // P  # 32 m-blocks
RB = 4  # m-blocks per batched row-min reduce

_cache = {}


def _build_nc():
    F32 = mybir.dt.float32
    F16 = mybir.dt.float16
    mn = mybir.AluOpType.min
    X = mybir.AxisListType.X

    nc = bacc.Bacc("TRN2", target_bir_lowering=False)
    ax_d = nc.declare_dram_parameter("ax", [KA, V], mybir.dt.bfloat16, isOutput=False)
    ay_d = nc.declare_dram_parameter("ay", [KA, V], mybir.dt.bfloat16, isOutput=False)
    idh_d = nc.declare_dram_parameter("identh", [P, P], F16, isOutput=False)
    ones_d = nc.declare_dram_parameter("ones", [P, 1], F32, isOutput=False)
    loss_d = nc.declare_dram_parameter("loss", [1, 1], F32, isOutput=True)

    with tile.TileContext(nc) as tc, ExitStack() as ctx:
        const = ctx.enter_context(tc.tile_pool(name="const", bufs=1))
        accs = ctx.enter_context(tc.tile_pool(name="accs", bufs=1))
        copies = ctx.enter_context(tc.tile_pool(name="copies", bufs=6))
        scratch = ctx.enter_context(tc.tile_pool(name="scratch", bufs=3))

        ax_sb = const.tile([KA, V], mybir.dt.bfloat16)
        ay_sb = const.tile([KA, V], mybir.dt.bfloat16)
        idh_sb = const.tile([P, P], F16)
        ones_sb = const.tile([P, 1], F32)
        nc.sync.dma_start(ax_sb[:], ax_d[:])
        nc.sync.dma_start(ay_sb[:], ay_d[:])
        nc.sync.dma_start(idh_sb[:], idh_d[:])
        nc.sync.dma_start(ones_sb[:], ones_d[:])

        cacc = [accs.tile([P, GRP], F16, name=f"cacc{g}") for g in range(NG)]
        fold4 = accs.tile([P, RB * NMM], F16, name="fold4")
        rowmin = accs.tile([P, MB], F32, name="rowmin")
        colmin = accs.tile([P, MB], F32, name="colmin")

        with tc.tile_pool(name="psum", bufs=2, space="PSUM") as psum:
            for m in range(MB):
                lhsT = ax_sb[:, m * P : (m + 1) * P]
                cpy = []
                for g in range(NG):
                    pst = psum.tile([P, GRP], F32, name=f"ps{g}", tag="ps")
                    for j in range(GRP // NMM):
                        c0 = g * GRP + j * NMM
                        nc.tensor.matmul(
                            pst[:, j * NMM : (j + 1) * NMM],
                            lhsT,
                            ay_sb[:, c0 : c0 + NMM],
                            start=True,
                            stop=True,
                        )
                    ct = copies.tile([P, GRP], F16, name=f"c{g}", tag=f"c{g}")
                    nc.scalar.copy(ct[:], pst[:])
                    cpy.append(ct)

                # col-direction running mins (fp16 2x TT)
                for g in range(NG):
                    if m == 0:
                        nc.vector.tensor_copy(cacc[g][:], cpy[g][:])
                    else:
                        nc.vector.tensor_tensor(cacc[g][:], cpy[g][:], cacc[g][:], mn)

                # row-direction fold tree: 4096 -> 2048 -> 1024 -> 512
                sc = scratch.tile([P, GRP], F16, name="sc", tag="sc")
                nc.vector.tensor_tensor(sc[:], cpy[0][:], cpy[1][:], mn)
                nc.vector.tensor_tensor(
                    sc[:, : GRP // 2], sc[:, : GRP // 2], sc[:, GRP // 2 :], mn
                )
                r = m % RB
                nc.vector.tensor_tensor(
                    fold4[:, r * NMM : (r + 1) * NMM],
                    sc[:, : GRP // 4],
                    sc[:, GRP // 4 : GRP // 2],
                    mn,
                )
                if r == RB - 1:
                    nc.vector.tensor_reduce(
                        rowmin[:, m - RB + 1 : m + 1],
                        fold4[:].rearrange("p (a b) -> p a b", a=RB),
                        axis=X,
                        op=mn,
                    )

        # Epilogue: transpose col accumulators -> per-w mins.
        with tc.tile_pool(name="psum_ep", bufs=4, space="PSUM") as psum_ep:
            for g in range(NG):
                for q in range(4):
                    tp = psum_ep.tile([P, 4 * P], F16, name="tp", tag="tp")
                    for k in range(4):
                        b = q * 4 + k
                        nc.tensor.transpose(
                            tp[:, k * P : (k + 1) * P],
                            cacc[g][:, b * P : (b + 1) * P],
                            idh_sb[:],
                        )
                    idx = g * 16 + q * 4
                    nc.vector.tensor_reduce(
                        colmin[:, idx : idx + 4],
                        tp[:].rearrange("p (a b) -> p a b", a=4),
                        axis=X,
                        op=mn,
                    )

            # relu (tiny negative sq from roundoff), sqrt + fused sum
            nc.vector.tensor_scalar_max(rowmin[:], rowmin[:], 0.0)
            nc.vector.tensor_scalar_max(colmin[:], colmin[:], 0.0)
            sa = accs.tile([P, 1], F32, name="sa")
            sb_ = accs.tile([P, 1], F32, name="sb_")
            nc.scalar.activation(
                rowmin[:], rowmin[:], mybir.ActivationFunctionType.Sqrt,
                accum_out=sa[:],
            )
            nc.scalar.activation(
                colmin[:], colmin[:], mybir.ActivationFunctionType.Sqrt,
                accum_out=sb_[:],
            )
            stot = accs.tile([P, 1], F32, name="stot")
            nc.vector.tensor_add(stot[:], sa[:], sb_[:])
            fin = psum_ep.tile([1, 1], F32, name="fin")
            nc.tensor.matmul(fin[:], stot[:], ones_sb[:], start=True, stop=True)
            res = accs.tile([1, 1], F32, name="res")
            nc.scalar.mul(res[:], fin[:], 1.0 / V)
            nc.sync.dma_start(loss_d[:], res[:])

    nc.finalize()
    return nc


def _augment(x, y):
    """x, y: (V, 3) fp32 -> AX, AY [13, V] bf16 hi/lo-split gram operands."""
    f32 = np.float32
    yy = (-2.0 * y).astype(f32)
    xh = x.astype(BF16)
    xl = (x - xh.astype(f32)).astype(BF16)
    yh = yy.astype(BF16)
    yl = (yy - yh.astype(f32)).astype(BF16)
    x2 = np.einsum("vc,vc->v", x.astype(np.float64), x.astype(np.float64)).astype(f32)
    y2 = np.einsum("vc,vc->v", y.astype(np.float64), y.astype(np.float64)).astype(f32)
    x2h = x2.astype(BF16)
    x2l = (x2 - x2h.astype(f32)).astype(BF16)
    y2h = y2.astype(BF16)
    y2l = (y2 - y2h.astype(f32)).astype(BF16)
    one = np.ones(V, dtype=BF16)
    ax = np.stack(
        [xh[:, 0], xh[:, 1], xh[:, 2],
         xh[:, 0], xh[:, 1], xh[:, 2],
         xl[:, 0], xl[:, 1], xl[:, 2],
         x2h, x2l, one, one]
    )
    ay = np.stack(
        [yh[:, 0], yh[:, 1], yh[:, 2],
         yl[:, 0], yl[:, 1], yl[:, 2],
         yh[:, 0], yh[:, 1], yh[:, 2],
         one, one, y2h, y2l]
    )
    return ax, ay


def kernel(x, y):
    x = np.asarray(x, dtype=np.float32)
    y = np.asarray(y, dtype=np.float32)
    n = x.shape[0]
    assert x.shape == (n, V, 3) and y.shape == (n, V, 3) and n == 8

    if "nc" not in _cache:
        _cache["nc"] = _build_nc()
    nc = _cache["nc"]

    identh = np.eye(P, dtype=np.float16)
    ones = np.ones((P, 1), dtype=np.float32)
    in_maps = []
    for i in range(n):
        ax, ay = _augment(x[i], y[i])
        in_maps.append({"ax": ax, "ay": ay, "identh": identh, "ones": ones})

    res = run_bass_kernel_spmd(
        nc, in_maps, list(range(n)), trace=_cache.get("trace", False)
    )
    _cache["last"] = res
    vals = [
        np.asarray(res.results[i]["loss"], dtype=np.float32).reshape(())
        for i in range(n)
    ]
    return np.float32(np.mean(vals))
